# revision 19
# baseline (speedup 1.0000x reference)
"""Trainium2 Bass kernel for a pre-norm transformer encoder block.

Problem: B=2, T=2048, C=1024, H=16 heads of 64, GELU FFN (4C), fp32.

Sharding: data-parallel over (batch, query-slice): 8 cores, core c handles
batch b=c//4 and token rows [(c%4)*512, (c%4+1)*512). Each core computes
LN1 + Q/K/V projections only for its OWN 512-token slice; K^T and the
ones-augmented V are exchanged among the 4 cores of a batch group with
pipelined DRAM AllGather collectives (K halves first so scores can start
before V lands). Attention/out-proj/FFN run on the core's 512 query rows.

All matmul operands are bf16 (fp32 PSUM accumulation); LN/softmax/residual
arithmetic stays fp32. Activations stay feature-major through the matmul
chain; PE transposes convert to/from token-major for LayerNorm. Softmax
runs over the partition axis with an ones-augmented V matmul producing
denominators for free; 1/denom via fast DVE reciprocal is broadcast across
partitions with a tiny ones-matmul on the PE (gpsimd stays free for the
collectives).
"""

import os
import sys

sys.path.insert(0, "/opt/trn_rl_repo")

import numpy as np

DBG = os.environ.get("BASSDBG", "0") == "1"

import concourse.bass as bass
import concourse.mybir as mybir
import concourse.tile as tile
from concourse import bacc, bass_utils
from concourse.masks import make_identity

P = 128
B, T, C, H = 2, 2048, 1024, 16
HS = C // H  # 64
F = 4 * C  # 4096
NQ = 512  # token rows per core
CC = C // P  # 8
FC = F // P  # 32
GT = NQ // P  # 4 token tiles per slice
VW = 2 * (HS + 1)  # 130: ones-augmented V columns per head pair
EPS = 1e-5

f32 = mybir.dt.float32
bfh = mybir.dt.bfloat16
AF = mybir.ActivationFunctionType
Alu = mybir.AluOpType

GROUPS = [[0, 1, 2, 3], [4, 5, 6, 7]]


def build_program():
    nc = bacc.Bacc("TRN2", target_bir_lowering=False, debug=False, num_devices=8)

    xq_d = nc.dram_tensor("xq", [NQ, C], f32, kind="ExternalInput").ap()
    wq_d = nc.dram_tensor("wq", [C, C], bfh, kind="ExternalInput").ap()
    wk_d = nc.dram_tensor("wk", [C, C], bfh, kind="ExternalInput").ap()
    wv_d = nc.dram_tensor("wv", [C, C], bfh, kind="ExternalInput").ap()
    wp_d = nc.dram_tensor("wp", [C, C], bfh, kind="ExternalInput").ap()
    w1_d = nc.dram_tensor("w1p", [FC, P, CC, P], bfh, kind="ExternalInput").ap()
    w2_d = nc.dram_tensor("w2p", [CC, P, FC, P], bfh, kind="ExternalInput").ap()
    bias_names = ["l1w", "l1b", "bk", "bq", "bv", "bp", "l2w", "l2b", "b2"]
    bias_d = {
        n: nc.dram_tensor(n, [C], f32, kind="ExternalInput").ap() for n in bias_names
    }
    b1_d = nc.dram_tensor("b1", [F], f32, kind="ExternalInput").ap()
    y_d = nc.dram_tensor("y", [NQ, C], f32, kind="ExternalOutput").ap()
    dbg = {}
    if DBG:
        dbg["xnq"] = nc.dram_tensor("d_xnq", [P, CC, NQ], bfh, kind="ExternalOutput").ap()
        dbg["qt"] = nc.dram_tensor("d_qt", [P, CC, NQ], bfh, kind="ExternalOutput").ap()
        dbg["kg"] = nc.dram_tensor("d_kg", [2, 4, GT, P, NQ], bfh, kind="ExternalOutput").ap()
        dbg["vg"] = nc.dram_tensor("d_vg", [2, 4, GT, P, 8 * (HS + 1)], bfh, kind="ExternalOutput").ap()
        dbg["ot"] = nc.dram_tensor("d_ot", [P, CC, NQ], bfh, kind="ExternalOutput").ap()
        dbg["outq"] = nc.dram_tensor("d_outq", [P, GT, C], f32, kind="ExternalOutput").ap()

    with tile.TileContext(nc) as tc:
        from contextlib import ExitStack

        with ExitStack() as top:
            const = top.enter_context(tc.tile_pool(name="const", bufs=1))
            ident = const.tile([P, P], f32)
            make_identity(nc, ident[:])
            ident_h = const.tile([P, P], bfh, tag="ident_h")
            make_identity(nc, ident_h[:])
            eps_t = const.tile([P, 1], f32, tag="eps")
            nc.vector.memset(eps_t[:], EPS)
            ones_f = const.tile([P, 1], f32, tag="ones_f")
            nc.vector.memset(ones_f[:], 1.0)
            ones_r = const.tile([P, 1], bfh, tag="ones_r")
            nc.vector.tensor_copy(ones_r[:], ones_f[:])
            onesw_f = const.tile([1, HS], f32, tag="onesw_f")
            nc.vector.memset(onesw_f[:], 1.0)
            onesw = const.tile([1, HS], bfh, tag="onesw")
            nc.vector.tensor_copy(onesw[:], onesw_f[:])

            # ---- bias loads: contiguous [o, P] staging + PE transpose ----
            bias_t = {}
            with ExitStack() as bctx:
                bsp = bctx.enter_context(tc.tile_pool(name="bstage", bufs=2))
                btp = bctx.enter_context(
                    tc.tile_pool(name="btp", bufs=2, space="PSUM")
                )
                for n in bias_names:
                    st = bsp.tile([CC, P], f32, tag="bst")
                    nc.sync.dma_start(st[:], bias_d[n].rearrange("(o p) -> o p", p=P))
                    pm = btp.tile([P, CC], f32, tag="btr")
                    nc.tensor.matmul(
                        pm[:], st[:], ident[0:CC, 0:CC], start=True, stop=True
                    )
                    bt = const.tile([P, CC], f32, tag=f"bias_{n}")
                    nc.scalar.copy(bt[:], pm[:])
                    bias_t[n] = bt
                st = bsp.tile([FC, P], f32, tag="bst32")
                nc.sync.dma_start(st[:], b1_d.rearrange("(o p) -> o p", p=P))
                pm = btp.tile([P, FC], f32, tag="btr32")
                nc.tensor.matmul(
                    pm[:], st[:], ident[0:FC, 0:FC], start=True, stop=True
                )
                b1_t = const.tile([P, FC], f32, tag="bias_b1")
                nc.scalar.copy(b1_t[:], pm[:])

            dramp = top.enter_context(tc.tile_pool(name="dscratch", bufs=1, space="DRAM"))
            # combined K+V bounce buffers per half h: [i, P, 1032]
            #   cols 0:512   = K^T block fc = h*4+i (own tokens)
            #   cols 512:1032 = ones-augmented V for token tile i, heads of half h
            KVW = NQ + 8 * (HS + 1)  # 1032
            kv_in = [
                dramp.tile([GT, P, KVW], bfh, name=f"kvin{h_}", tag=f"kvin{h_}")
                for h_ in range(2)
            ]
            kv_g = [
                dramp.tile([4, GT, P, KVW], bfh, name=f"kvg{h_}", tag=f"kvg{h_}")
                for h_ in range(2)
            ]

            res = top.enter_context(tc.tile_pool(name="resident", bufs=1))
            QT_t = res.tile([P, CC, NQ], bfh, tag="QT")

            # ---------------- Phase A: LN1 + Q/K/V projections ----------------
            with ExitStack() as ph:
                lnp = ph.enter_context(tc.tile_pool(name="lnp", bufs=4))
                trp = ph.enter_context(tc.tile_pool(name="trp", bufs=3, space="PSUM"))
                mmp = ph.enter_context(tc.tile_pool(name="mmpA", bufs=4, space="PSUM"))
                evp = ph.enter_context(tc.tile_pool(name="evpA", bufs=3))
                wgp = ph.enter_context(tc.tile_pool(name="wgp", bufs=1))

                xnq = res.tile([P, CC, NQ], bfh, tag="xnT")

                wk_t = wgp.tile([P, CC, C], bfh, tag="wk")
                nc.sync.dma_start(wk_t[:], wk_d.rearrange("(o p) f -> p o f", p=P))
                wv_t = wgp.tile([P, CC, C], bfh, tag="wv")
                nc.sync.dma_start(wv_t[:], wv_d.rearrange("(o p) f -> p o f", p=P))
                wq_t = wgp.tile([P, CC, C], bfh, tag="wq")
                nc.sync.dma_start(wq_t[:], wq_d.rearrange("(o p) f -> p o f", p=P))

                # LN1 over own slice -> xnq (feature-major, bf16, ln1 applied)
                inv_c = 1.0 / C
                for tt in range(GT):
                    xt = lnp.tile([P, C], f32, tag="ln_x")
                    nc.sync.dma_start(xt[:], xq_d[tt * P : (tt + 1) * P, :])
                    stats = lnp.tile([P, 2, 6], f32, tag="ln_st6")
                    nc.vector.bn_stats(stats[:, 0, :], xt[:, 0:512])
                    nc.vector.bn_stats(stats[:, 1, :], xt[:, 512:1024])
                    mv = lnp.tile([P, 2], f32, tag="ln_mv")
                    nc.vector.bn_aggr(mv[:], stats[:])
                    st = lnp.tile([P, 1], f32, tag="ln_sd")
                    nc.scalar.activation(
                        st[:], mv[:, 1:2], AF.Sqrt, bias=eps_t[:]
                    )
                    rs = lnp.tile([P, 1], f32, tag="ln_rs")
                    nc.vector.reciprocal(rs[:], st[:])
                    xn = lnp.tile([P, C], bfh, tag="ln_xn")
                    nc.vector.tensor_scalar(
                        xn[:], xt[:], mv[:, 0:1], rs[:],
                        op0=Alu.subtract, op1=Alu.mult,
                    )
                    for cc in range(CC):
                        pt = trp.tile([P, P], bfh, tag="ln_tr")
                        nc.tensor.transpose(pt[:], xn[:, cc * P : (cc + 1) * P], ident_h[:])
                        nc.vector.tensor_scalar(
                            xnq[:, cc, tt * P : (tt + 1) * P],
                            pt[:],
                            bias_t["l1w"][:, cc : cc + 1],
                            bias_t["l1b"][:, cc : cc + 1],
                            op0=Alu.mult,
                            op1=Alu.add,
                        )

                # K projection (own slice) for half h_ -> kv_in cols 0:512
                def k_half(h_):
                    for fi in range(4):
                        fc = h_ * 4 + fi
                        pm = mmp.tile([P, NQ], f32, tag="mmA")
                        for cc in range(CC):
                            nc.tensor.matmul(
                                pm[:],
                                wk_t[:, cc, fc * P : (fc + 1) * P],
                                xnq[:, cc, :],
                                start=(cc == 0),
                                stop=(cc == CC - 1),
                            )
                        ev = evp.tile([P, NQ], bfh, tag="kev")
                        nc.scalar.activation(
                            ev[:], pm[:], AF.Identity,
                            bias=bias_t["bk"][:, fc : fc + 1],
                        )
                        nc.sync.dma_start(kv_in[h_][fi, :, 0:NQ], ev[:])

                # V projection (own slice), ones-augmented -> kv_in cols 512:1032
                def v_half(h_):
                    for tt in range(GT):
                        pm = mmp.tile([P, NQ], f32, tag="mmA")
                        for cc in range(CC):
                            nc.tensor.matmul(
                                pm[:],
                                xnq[:, cc, tt * P : (tt + 1) * P],
                                wv_t[:, cc, h_ * 512 : (h_ + 1) * 512],
                                start=(cc == 0),
                                stop=(cc == CC - 1),
                            )
                        ev = evp.tile([P, 8, HS + 1], bfh, tag="vev")
                        nc.scalar.copy(
                            ev[:, :, 0:HS],
                            pm[:].rearrange("p (h d) -> p h d", d=HS),
                        )
                        nc.vector.memset(ev[:, :, HS : HS + 1], 1.0)
                        nc.sync.dma_start(kv_in[h_][tt, :, NQ:KVW], ev[:])

                def fire(buf_in, buf_out):
                    nc.gpsimd.collective_compute(
                        "AllGather",
                        Alu.bypass,
                        replica_groups=GROUPS,
                        ins=[buf_in.opt()],
                        outs=[buf_out.opt()],
                    )

                k_half(0)
                v_half(0)
                fire(kv_in[0], kv_g[0])
                k_half(1)
                v_half(1)
                fire(kv_in[1], kv_g[1])

                # Q projection from own slice
                for fc in range(CC):
                    pm = mmp.tile([P, NQ], f32, tag="mmA")
                    for cc in range(CC):
                        nc.tensor.matmul(
                            pm[:],
                            wq_t[:, cc, fc * P : (fc + 1) * P],
                            xnq[:, cc, :],
                            start=(cc == 0),
                            stop=(cc == CC - 1),
                        )
                    nc.scalar.activation(
                        QT_t[:, fc, :], pm[:], AF.Identity,
                        bias=bias_t["bq"][:, fc : fc + 1],
                    )
                if DBG:
                    nc.sync.dma_start(dbg["xnq"], xnq[:])
                    nc.sync.dma_start(dbg["qt"], QT_t[:])

            # ---------------- Phase B: attention (per head pair) ----------------
            resC = top.enter_context(tc.tile_pool(name="resC", bufs=1))
            wp_t = resC.tile([P, CC, C], bfh, tag="wp")
            nc.sync.dma_start(wp_t[:], wp_d.rearrange("(o p) f -> p o f", p=P))
            xq_t = resC.tile([P, GT, C], f32, tag="xqt")
            nc.sync.dma_start(xq_t[:], xq_d.rearrange("(q p) c -> p q c", p=P))

            with ExitStack() as ph:
                kp = ph.enter_context(tc.tile_pool(name="kp", bufs=2))
                vap = ph.enter_context(tc.tile_pool(name="vap", bufs=6))
                ep = ph.enter_context(tc.tile_pool(name="ep", bufs=16))
                sp = ph.enter_context(tc.tile_pool(name="sp", bufs=2, space="PSUM"))
                op_ = ph.enter_context(tc.tile_pool(name="op", bufs=2, space="PSUM"))
                rbp = ph.enter_context(tc.tile_pool(name="rbp", bufs=2, space="PSUM"))
                npool = ph.enter_context(tc.tile_pool(name="npool", bufs=4))
                OT_t = res.tile([P, CC, NQ], bfh, tag="OT")

                for fc in range(CC):  # head pair (2*fc, 2*fc+1)
                    h_ = fc // 4
                    fi = fc % 4
                    kT_pair = kp.tile([P, T], bfh, tag="kT")
                    for slot in range(4):
                        nc.sync.dma_start(
                            kT_pair[:, slot * NQ : (slot + 1) * NQ],
                            kv_g[h_][slot, fi, :, 0:NQ],
                        )
                    O0 = op_.tile([HS + 1, NQ], f32, tag="Oacc")
                    O1 = op_.tile([HS + 1, NQ], f32, tag="Oacc")
                    SKEW = 4
                    pend = {}
                    for step in range(16 + SKEW):
                        if step < 16:
                            tc_i = step
                            s01 = sp.tile([P, 2 * NQ], f32, tag="sc")
                            nc.tensor.matmul(
                                s01[:, 0:NQ],
                                kT_pair[0:64, tc_i * P : (tc_i + 1) * P],
                                QT_t[0:64, fc, :],
                                start=True,
                                stop=True,
                            )
                            nc.tensor.matmul(
                                s01[:, NQ : 2 * NQ],
                                kT_pair[64:128, tc_i * P : (tc_i + 1) * P],
                                QT_t[64:128, fc, :],
                                start=True,
                                stop=True,
                                tile_position=(64, 0),
                            )
                            e01 = ep.tile([P, 2 * NQ], bfh, tag="e01")
                            nc.scalar.activation(e01[:], s01[:], AF.Exp, scale=C**-0.5)
                            va = vap.tile([P, VW], bfh, tag="va")
                            nc.sync.dma_start(
                                va[:],
                                kv_g[h_][tc_i // GT, tc_i % GT][
                                    :, NQ + fi * VW : NQ + (fi + 1) * VW
                                ],
                            )
                            pend[tc_i] = (e01, va)
                        if step >= SKEW:
                            tc_j = step - SKEW
                            e01, va = pend.pop(tc_j)
                            nc.tensor.matmul(
                                O0[:], va[:, 0 : HS + 1], e01[:, 0:NQ],
                                start=(tc_j == 0), stop=(tc_j == 15),
                            )
                            nc.tensor.matmul(
                                O1[:], va[:, HS + 1 : VW], e01[:, NQ : 2 * NQ],
                                start=(tc_j == 0), stop=(tc_j == 15),
                            )
                    for Oacc, col0 in ((O0, 0), (O1, 64)):
                        den = npool.tile([1, NQ], f32, tag="den")
                        nc.vector.tensor_copy(den[:], Oacc[HS : HS + 1, :])
                        rc = npool.tile([1, NQ], f32, tag="rc")
                        nc.vector.reciprocal_approx_fast(rc[:], den[:])
                        rch = npool.tile([1, NQ], bfh, tag="rch")
                        nc.vector.tensor_copy(rch[:], rc[:])
                        rb = rbp.tile([HS, NQ], f32, tag="rb")
                        nc.tensor.matmul(rb[:], onesw[:], rch[:], start=True, stop=True)
                        rbs = npool.tile([HS, NQ], f32, tag="rbs")
                        nc.vector.tensor_copy(rbs[:], rb[:])
                        dst = OT_t[col0 : col0 + HS, fc, :]
                        nc.vector.tensor_tensor(dst, Oacc[0:HS, :], rbs[:], op=Alu.mult)
                        nc.vector.tensor_scalar_add(
                            dst, dst, bias_t["bv"][col0 : col0 + HS, fc : fc + 1]
                        )

            if DBG:
                nc.sync.dma_start(dbg["ot"], OT_t[:])
                for h_ in range(2):
                    nc.sync.dma_start(dbg["kg"][h_], kv_g[h_][:, :, :, 0:NQ])
                    nc.sync.dma_start(dbg["vg"][h_], kv_g[h_][:, :, :, NQ:KVW])

            # ------------- Phase C: out-proj + residual + LN2 -------------
            outq_t = resC.tile([P, GT, C], f32, tag="outq")
            onT_t = resC.tile([P, CC, NQ], bfh, tag="onT")
            with ExitStack() as ph:
                lnp = ph.enter_context(tc.tile_pool(name="lnpC", bufs=2))
                trp = ph.enter_context(tc.tile_pool(name="trpC", bufs=3, space="PSUM"))
                mmp = ph.enter_context(tc.tile_pool(name="mmpC", bufs=3, space="PSUM"))
                evp = ph.enter_context(tc.tile_pool(name="evpC", bufs=3))

                for co in range(CC):
                    pm = mmp.tile([P, NQ], f32, tag="mmC")
                    for ci in range(CC):
                        nc.tensor.matmul(
                            pm[:],
                            wp_t[:, ci, co * P : (co + 1) * P],
                            OT_t[:, ci, :],
                            start=(ci == 0),
                            stop=(ci == CC - 1),
                        )
                    saT = evp.tile([P, NQ], f32, tag="saT")
                    nc.scalar.activation(
                        saT[:], pm[:], AF.Identity,
                        bias=bias_t["bp"][:, co : co + 1],
                    )
                    for qt in range(GT):
                        pt = trp.tile([P, P], f32, tag="trC")
                        nc.tensor.transpose(
                            pt[:], saT[:, qt * P : (qt + 1) * P], ident[:]
                        )
                        nc.vector.tensor_tensor(
                            outq_t[:, qt, co * P : (co + 1) * P],
                            pt[:],
                            xq_t[:, qt, co * P : (co + 1) * P],
                            op=Alu.add,
                        )
                if DBG:
                    nc.sync.dma_start(dbg["outq"], outq_t[:])
                # LN2 (token-major, input already in SBUF) -> feature-major onT
                for qt in range(GT):
                    xt = outq_t[:, qt, :]
                    stats = lnp.tile([P, 2, 6], f32, tag="ln_st6")
                    nc.vector.bn_stats(stats[:, 0, :], xt[:, 0:512])
                    nc.vector.bn_stats(stats[:, 1, :], xt[:, 512:1024])
                    mv = lnp.tile([P, 2], f32, tag="ln_mv")
                    nc.vector.bn_aggr(mv[:], stats[:])
                    st = lnp.tile([P, 1], f32, tag="ln_sd")
                    nc.scalar.activation(st[:], mv[:, 1:2], AF.Sqrt, bias=eps_t[:])
                    rs = lnp.tile([P, 1], f32, tag="ln_rs")
                    nc.vector.reciprocal(rs[:], st[:])
                    xn = lnp.tile([P, C], bfh, tag="ln_xn")
                    nc.gpsimd.tensor_scalar(
                        xn[:], xt, mv[:, 0:1], rs[:],
                        op0=Alu.subtract, op1=Alu.mult,
                    )
                    for cc in range(CC):
                        pt = trp.tile([P, P], bfh, tag="trC")
                        nc.tensor.transpose(
                            pt[:], xn[:, cc * P : (cc + 1) * P], ident_h[:]
                        )
                        nc.vector.tensor_scalar(
                            onT_t[:, cc, qt * P : (qt + 1) * P],
                            pt[:],
                            bias_t["l2w"][:, cc : cc + 1],
                            bias_t["l2b"][:, cc : cc + 1],
                            op0=Alu.mult,
                            op1=Alu.add,
                        )

            # ---------------- Phase D: FFN ----------------
            with ExitStack() as ph:
                w1p = ph.enter_context(tc.tile_pool(name="w1p", bufs=3))
                w2p = ph.enter_context(tc.tile_pool(name="w2p", bufs=2))
                hp = ph.enter_context(tc.tile_pool(name="hp", bufs=1))
                mmph = ph.enter_context(tc.tile_pool(name="mmph", bufs=3, space="PSUM"))
                mmpy = ph.enter_context(tc.tile_pool(name="mmpy", bufs=2, space="PSUM"))
                trp = ph.enter_context(tc.tile_pool(name="trpD", bufs=2, space="PSUM"))
                evp = ph.enter_context(tc.tile_pool(name="evpD", bufs=3))
                finp = ph.enter_context(tc.tile_pool(name="finp", bufs=1))

                hT_t = hp.tile([P, FC, NQ], bfh, tag="hT")
                final_t = finp.tile([P, GT, C], f32, tag="final")

                for fc in range(FC):
                    w1c = w1p.tile([P, CC, P], bfh, tag="w1c")
                    nc.sync.dma_start(w1c[:], w1_d[fc])
                    pm = mmph.tile([P, NQ], f32, tag="mmh")
                    for cc in range(CC):
                        nc.tensor.matmul(
                            pm[:],
                            w1c[:, cc, :],
                            onT_t[:, cc, :],
                            start=(cc == 0),
                            stop=(cc == CC - 1),
                        )
                    nc.scalar.activation(
                        hT_t[:, fc, :], pm[:], AF.Gelu, bias=b1_t[:, fc : fc + 1]
                    )

                for co in range(CC):
                    w2c = w2p.tile([P, FC, P], bfh, tag="w2c")
                    nc.sync.dma_start(w2c[:], w2_d[co])
                    pm = mmpy.tile([P, NQ], f32, tag="mmy")
                    for fc in range(FC):
                        nc.tensor.matmul(
                            pm[:],
                            w2c[:, fc, :],
                            hT_t[:, fc, :],
                            start=(fc == 0),
                            stop=(fc == FC - 1),
                        )
                    yT = evp.tile([P, NQ], f32, tag="yT")
                    nc.scalar.activation(
                        yT[:], pm[:], AF.Identity,
                        bias=bias_t["b2"][:, co : co + 1],
                    )
                    for qt in range(GT):
                        pt = trp.tile([P, P], f32, tag="trD")
                        nc.tensor.transpose(
                            pt[:], yT[:, qt * P : (qt + 1) * P], ident[:]
                        )
                        nc.vector.tensor_tensor(
                            final_t[:, qt, co * P : (co + 1) * P],
                            pt[:],
                            outq_t[:, qt, co * P : (co + 1) * P],
                            op=Alu.add,
                        )
                nc.sync.dma_start(
                    y_d.rearrange("(q p) c -> p q c", p=P), final_t[:]
                )

    nc.compile()
    return nc


_NC_CACHE = None


def _get_program():
    global _NC_CACHE
    if _NC_CACHE is None:
        _NC_CACHE = build_program()
    return _NC_CACHE


import ml_dtypes

BF16 = ml_dtypes.bfloat16


def _merge_heads(w):
    # [H, C, HS] -> [C, H*HS]
    return np.ascontiguousarray(
        np.transpose(np.asarray(w), (1, 0, 2)).reshape(C, C).astype(BF16)
    )


def make_in_maps(inputs):
    x = np.ascontiguousarray(np.asarray(inputs["x"], dtype=np.float32))
    w1 = np.asarray(inputs["W1"], np.float32).astype(BF16)
    w2 = np.asarray(inputs["W2"], np.float32).astype(BF16)
    shared = {
        "wq": _merge_heads(inputs["Wq"]),
        "wk": _merge_heads(inputs["Wk"]),
        "wv": _merge_heads(inputs["Wv"]),
        "wp": np.ascontiguousarray(np.asarray(inputs["Wp"], np.float32).astype(BF16)),
        "w1p": np.ascontiguousarray(
            w1.reshape(CC, P, FC, P).transpose(2, 1, 0, 3)
        ),
        "w2p": np.ascontiguousarray(
            w2.reshape(FC, P, CC, P).transpose(2, 1, 0, 3)
        ),
        "bq": np.asarray(inputs["bq"], np.float32).reshape(C).copy(),
        "bk": np.asarray(inputs["bk"], np.float32).reshape(C).copy(),
        "bv": np.asarray(inputs["bv"], np.float32).reshape(C).copy(),
        "bp": np.asarray(inputs["bp"], np.float32).copy(),
        "b1": np.asarray(inputs["b1"], np.float32).copy(),
        "b2": np.asarray(inputs["b2"], np.float32).copy(),
        "l1w": np.asarray(inputs["ln1_w"], np.float32).copy(),
        "l1b": np.asarray(inputs["ln1_b"], np.float32).copy(),
        "l2w": np.asarray(inputs["ln2_w"], np.float32).copy(),
        "l2b": np.asarray(inputs["ln2_b"], np.float32).copy(),
    }
    in_maps = []
    for c in range(8):
        b, qs = c // 4, c % 4
        m = dict(shared)
        m["xq"] = np.ascontiguousarray(x[b, qs * NQ : (qs + 1) * NQ])
        in_maps.append(m)
    return in_maps


def kernel(**inputs):
    in_maps = make_in_maps(inputs)
    nc = _get_program()
    res = bass_utils.run_bass_kernel_spmd(nc, in_maps, core_ids=list(range(8)))
    out = np.empty((B, T, C), np.float32)
    for c in range(8):
        b, qs = c // 4, c % 4
        out[b, qs * NQ : (qs + 1) * NQ] = res.results[c]["y"]
    return out


# revision 20
# speedup vs baseline: 1.1116x; 1.1116x over previous
"""Trainium2 Bass kernel for a pre-norm transformer encoder block.

Problem: B=2, T=2048, C=1024, H=16 heads of 64, GELU FFN (4C), fp32.

Sharding: data-parallel over (batch, query-slice): 8 cores, core c handles
batch b=c//4 and token rows [(c%4)*512, (c%4+1)*512). Each core computes
LN1 + Q/K/V projections only for its OWN 512-token slice; K^T and the
ones-augmented V are exchanged among the 4 cores of a batch group with
pipelined DRAM AllGather collectives (K halves first so scores can start
before V lands). Attention/out-proj/FFN run on the core's 512 query rows.

All matmul operands are bf16 (fp32 PSUM accumulation); LN/softmax/residual
arithmetic stays fp32. Activations stay feature-major through the matmul
chain; PE transposes convert to/from token-major for LayerNorm. Softmax
runs over the partition axis with an ones-augmented V matmul producing
denominators for free; 1/denom via fast DVE reciprocal is broadcast across
partitions with a tiny ones-matmul on the PE (gpsimd stays free for the
collectives).
"""

import os
import sys

sys.path.insert(0, "/opt/trn_rl_repo")

import numpy as np

DBG = os.environ.get("BASSDBG", "0") == "1"

import concourse.bass as bass
import concourse.mybir as mybir
import concourse.tile as tile
from concourse import bacc, bass_utils
from concourse.masks import make_identity

P = 128
B, T, C, H = 2, 2048, 1024, 16
HS = C // H  # 64
F = 4 * C  # 4096
NQ = 512  # token rows per core
CC = C // P  # 8
FC = F // P  # 32
GT = NQ // P  # 4 token tiles per slice
VW = 2 * (HS + 1)  # 130: ones-augmented V columns per head pair
EPS = 1e-5

f32 = mybir.dt.float32
bfh = mybir.dt.bfloat16
AF = mybir.ActivationFunctionType
Alu = mybir.AluOpType

GROUPS = [[0, 1, 2, 3], [4, 5, 6, 7]]


def build_program():
    nc = bacc.Bacc("TRN2", target_bir_lowering=False, debug=False, num_devices=8)

    xq_d = nc.dram_tensor("xq", [NQ, C], f32, kind="ExternalInput").ap()
    wq_d = nc.dram_tensor("wq", [C, C], bfh, kind="ExternalInput").ap()
    wk_d = nc.dram_tensor("wk", [C, C], bfh, kind="ExternalInput").ap()
    wv_d = nc.dram_tensor("wv", [C, C], bfh, kind="ExternalInput").ap()
    wp_d = nc.dram_tensor("wp", [C, C], bfh, kind="ExternalInput").ap()
    w1_d = nc.dram_tensor("w1p", [FC, P, CC, P], bfh, kind="ExternalInput").ap()
    w2_d = nc.dram_tensor("w2p", [CC, P, FC, P], bfh, kind="ExternalInput").ap()
    bias_names = ["l1w", "l1b", "bk", "bq", "bv", "bp", "l2w", "l2b", "b2"]
    bias_d = {
        n: nc.dram_tensor(n, [C], f32, kind="ExternalInput").ap() for n in bias_names
    }
    b1_d = nc.dram_tensor("b1", [F], f32, kind="ExternalInput").ap()
    y_d = nc.dram_tensor("y", [NQ, C], f32, kind="ExternalOutput").ap()
    dbg = {}
    if DBG:
        dbg["xnq"] = nc.dram_tensor("d_xnq", [P, CC, NQ], bfh, kind="ExternalOutput").ap()
        dbg["qt"] = nc.dram_tensor("d_qt", [P, CC, NQ], bfh, kind="ExternalOutput").ap()
        dbg["kg"] = nc.dram_tensor("d_kg", [2, 4, GT, P, NQ], bfh, kind="ExternalOutput").ap()
        dbg["vg"] = nc.dram_tensor("d_vg", [2, 4, GT, P, 8 * (HS + 1)], bfh, kind="ExternalOutput").ap()
        dbg["ot"] = nc.dram_tensor("d_ot", [P, CC, NQ], bfh, kind="ExternalOutput").ap()
        dbg["outq"] = nc.dram_tensor("d_outq", [P, GT, C], f32, kind="ExternalOutput").ap()

    with tile.TileContext(nc) as tc:
        from contextlib import ExitStack

        with ExitStack() as top:
            const = top.enter_context(tc.tile_pool(name="const", bufs=1))
            ident = const.tile([P, P], f32)
            make_identity(nc, ident[:])
            ident_h = const.tile([P, P], bfh, tag="ident_h")
            make_identity(nc, ident_h[:])
            eps_t = const.tile([P, 1], f32, tag="eps")
            nc.vector.memset(eps_t[:], EPS)
            ones_f = const.tile([P, 1], f32, tag="ones_f")
            nc.vector.memset(ones_f[:], 1.0)
            ones_r = const.tile([P, 1], bfh, tag="ones_r")
            nc.vector.tensor_copy(ones_r[:], ones_f[:])
            onesw_f = const.tile([1, HS], f32, tag="onesw_f")
            nc.vector.memset(onesw_f[:], 1.0)
            onesw = const.tile([1, HS], bfh, tag="onesw")
            nc.vector.tensor_copy(onesw[:], onesw_f[:])

            # ---- bias loads: contiguous [o, P] staging + PE transpose ----
            bias_t = {}
            with ExitStack() as bctx:
                bsp = bctx.enter_context(tc.tile_pool(name="bstage", bufs=2))
                btp = bctx.enter_context(
                    tc.tile_pool(name="btp", bufs=2, space="PSUM")
                )
                for n in bias_names:
                    st = bsp.tile([CC, P], f32, tag="bst")
                    nc.sync.dma_start(st[:], bias_d[n].rearrange("(o p) -> o p", p=P))
                    pm = btp.tile([P, CC], f32, tag="btr")
                    nc.tensor.matmul(
                        pm[:], st[:], ident[0:CC, 0:CC], start=True, stop=True
                    )
                    bt = const.tile([P, CC], f32, tag=f"bias_{n}")
                    nc.scalar.copy(bt[:], pm[:])
                    bias_t[n] = bt
                st = bsp.tile([FC, P], f32, tag="bst32")
                nc.sync.dma_start(st[:], b1_d.rearrange("(o p) -> o p", p=P))
                pm = btp.tile([P, FC], f32, tag="btr32")
                nc.tensor.matmul(
                    pm[:], st[:], ident[0:FC, 0:FC], start=True, stop=True
                )
                b1_t = const.tile([P, FC], f32, tag="bias_b1")
                nc.scalar.copy(b1_t[:], pm[:])

            dramp = top.enter_context(tc.tile_pool(name="dscratch", bufs=1, space="DRAM"))
            VH = 8 * (HS + 1)  # 520
            kT_in = [
                dramp.tile([GT, P, NQ], bfh, name=f"kTin{h_}", tag=f"kTin{h_}")
                for h_ in range(2)
            ]
            v_in = [
                dramp.tile([GT, P, VH], bfh, name=f"vin{h_}", tag=f"vin{h_}")
                for h_ in range(2)
            ]
            kT_g = [
                dramp.tile([4, GT, P, NQ], bfh, name=f"kTg{h_}", tag=f"kTg{h_}")
                for h_ in range(2)
            ]
            v_g = [
                dramp.tile([4, GT, P, VH], bfh, name=f"vg{h_}", tag=f"vg{h_}")
                for h_ in range(2)
            ]

            res = top.enter_context(tc.tile_pool(name="resident", bufs=1))
            QT_t = res.tile([P, CC, NQ], bfh, tag="QT")

            # ---------------- Phase A: LN1 + Q/K/V projections ----------------
            with ExitStack() as ph:
                lnp = ph.enter_context(tc.tile_pool(name="lnp", bufs=4))
                trp = ph.enter_context(tc.tile_pool(name="trp", bufs=3, space="PSUM"))
                mmp = ph.enter_context(tc.tile_pool(name="mmpA", bufs=4, space="PSUM"))
                evp = ph.enter_context(tc.tile_pool(name="evpA", bufs=3))
                wgp = ph.enter_context(tc.tile_pool(name="wgp", bufs=1))

                xnq = res.tile([P, CC, NQ], bfh, tag="xnT")

                wk_t = wgp.tile([P, CC, C], bfh, tag="wk")
                nc.sync.dma_start(wk_t[:], wk_d.rearrange("(o p) f -> p o f", p=P))
                wv_t = wgp.tile([P, CC, C], bfh, tag="wv")
                nc.sync.dma_start(wv_t[:], wv_d.rearrange("(o p) f -> p o f", p=P))
                wq_t = wgp.tile([P, CC, C], bfh, tag="wq")
                nc.sync.dma_start(wq_t[:], wq_d.rearrange("(o p) f -> p o f", p=P))

                # LN1 over own slice -> xnq (feature-major, bf16, ln1 applied)
                inv_c = 1.0 / C
                for tt in range(GT):
                    xt = lnp.tile([P, C], f32, tag="ln_x")
                    nc.sync.dma_start(xt[:], xq_d[tt * P : (tt + 1) * P, :])
                    stats = lnp.tile([P, 2, 6], f32, tag="ln_st6")
                    nc.vector.bn_stats(stats[:, 0, :], xt[:, 0:512])
                    nc.vector.bn_stats(stats[:, 1, :], xt[:, 512:1024])
                    mv = lnp.tile([P, 2], f32, tag="ln_mv")
                    nc.vector.bn_aggr(mv[:], stats[:])
                    st = lnp.tile([P, 1], f32, tag="ln_sd")
                    nc.scalar.activation(
                        st[:], mv[:, 1:2], AF.Sqrt, bias=eps_t[:]
                    )
                    rs = lnp.tile([P, 1], f32, tag="ln_rs")
                    nc.vector.reciprocal(rs[:], st[:])
                    xn = lnp.tile([P, C], bfh, tag="ln_xn")
                    nc.vector.tensor_scalar(
                        xn[:], xt[:], mv[:, 0:1], rs[:],
                        op0=Alu.subtract, op1=Alu.mult,
                    )
                    for cc in range(CC):
                        pt = trp.tile([P, P], bfh, tag="ln_tr")
                        nc.tensor.transpose(pt[:], xn[:, cc * P : (cc + 1) * P], ident_h[:])
                        nc.vector.tensor_scalar(
                            xnq[:, cc, tt * P : (tt + 1) * P],
                            pt[:],
                            bias_t["l1w"][:, cc : cc + 1],
                            bias_t["l1b"][:, cc : cc + 1],
                            op0=Alu.mult,
                            op1=Alu.add,
                        )

                # K projection (own slice) for half h_ -> kv_in cols 0:512
                def k_half(h_):
                    for fi in range(4):
                        fc = h_ * 4 + fi
                        pm = mmp.tile([P, NQ], f32, tag="mmA")
                        for cc in range(CC):
                            nc.tensor.matmul(
                                pm[:],
                                wk_t[:, cc, fc * P : (fc + 1) * P],
                                xnq[:, cc, :],
                                start=(cc == 0),
                                stop=(cc == CC - 1),
                            )
                        ev = evp.tile([P, NQ], bfh, tag="kev")
                        nc.scalar.activation(
                            ev[:], pm[:], AF.Identity,
                            bias=bias_t["bk"][:, fc : fc + 1],
                        )
                        nc.sync.dma_start(kT_in[h_][fi], ev[:])

                # V projection (own slice), ones-augmented -> kv_in cols 512:1032
                def v_half(h_):
                    for tt in range(GT):
                        pm = mmp.tile([P, NQ], f32, tag="mmA")
                        for cc in range(CC):
                            nc.tensor.matmul(
                                pm[:],
                                xnq[:, cc, tt * P : (tt + 1) * P],
                                wv_t[:, cc, h_ * 512 : (h_ + 1) * 512],
                                start=(cc == 0),
                                stop=(cc == CC - 1),
                            )
                        ev = evp.tile([P, 8, HS + 1], bfh, tag="vev")
                        nc.scalar.copy(
                            ev[:, :, 0:HS],
                            pm[:].rearrange("p (h d) -> p h d", d=HS),
                        )
                        nc.vector.memset(ev[:, :, HS : HS + 1], 1.0)
                        nc.sync.dma_start(v_in[h_][tt], ev[:])

                def fire(buf_in, buf_out):
                    nc.gpsimd.collective_compute(
                        "AllGather",
                        Alu.bypass,
                        replica_groups=GROUPS,
                        ins=[buf_in.opt()],
                        outs=[buf_out.opt()],
                    )

                k_half(0)
                fire(kT_in[0], kT_g[0])
                v_half(0)
                fire(v_in[0], v_g[0])
                k_half(1)
                fire(kT_in[1], kT_g[1])
                v_half(1)
                fire(v_in[1], v_g[1])

                # Q projection from own slice
                for fc in range(CC):
                    pm = mmp.tile([P, NQ], f32, tag="mmA")
                    for cc in range(CC):
                        nc.tensor.matmul(
                            pm[:],
                            wq_t[:, cc, fc * P : (fc + 1) * P],
                            xnq[:, cc, :],
                            start=(cc == 0),
                            stop=(cc == CC - 1),
                        )
                    nc.scalar.activation(
                        QT_t[:, fc, :], pm[:], AF.Identity,
                        bias=bias_t["bq"][:, fc : fc + 1],
                    )
                if DBG:
                    nc.sync.dma_start(dbg["xnq"], xnq[:])
                    nc.sync.dma_start(dbg["qt"], QT_t[:])

            # ---------------- Phase B: attention (per head pair) ----------------
            resC = top.enter_context(tc.tile_pool(name="resC", bufs=1))
            wp_t = resC.tile([P, CC, C], bfh, tag="wp")
            nc.sync.dma_start(wp_t[:], wp_d.rearrange("(o p) f -> p o f", p=P))
            xq_t = resC.tile([P, GT, C], f32, tag="xqt")
            nc.sync.dma_start(xq_t[:], xq_d.rearrange("(q p) c -> p q c", p=P))

            with ExitStack() as ph:
                kp = ph.enter_context(tc.tile_pool(name="kp", bufs=2))
                vap = ph.enter_context(tc.tile_pool(name="vap", bufs=6))
                ep = ph.enter_context(tc.tile_pool(name="ep", bufs=16))
                sp = ph.enter_context(tc.tile_pool(name="sp", bufs=2, space="PSUM"))
                op_ = ph.enter_context(tc.tile_pool(name="op", bufs=2, space="PSUM"))
                rbp = ph.enter_context(tc.tile_pool(name="rbp", bufs=2, space="PSUM"))
                npool = ph.enter_context(tc.tile_pool(name="npool", bufs=4))
                OT_t = res.tile([P, CC, NQ], bfh, tag="OT")

                for fc in range(CC):  # head pair (2*fc, 2*fc+1)
                    h_ = fc // 4
                    fi = fc % 4
                    kT_pair = kp.tile([P, T], bfh, tag="kT")
                    for slot in range(4):
                        nc.sync.dma_start(
                            kT_pair[:, slot * NQ : (slot + 1) * NQ],
                            kT_g[h_][slot, fi],
                        )
                    O0 = op_.tile([HS + 1, NQ], f32, tag="Oacc")
                    O1 = op_.tile([HS + 1, NQ], f32, tag="Oacc")
                    SKEW = 4
                    pend = {}
                    for step in range(16 + SKEW):
                        if step < 16:
                            tc_i = step
                            s01 = sp.tile([P, 2 * NQ], f32, tag="sc")
                            nc.tensor.matmul(
                                s01[:, 0:NQ],
                                kT_pair[0:64, tc_i * P : (tc_i + 1) * P],
                                QT_t[0:64, fc, :],
                                start=True,
                                stop=True,
                            )
                            nc.tensor.matmul(
                                s01[:, NQ : 2 * NQ],
                                kT_pair[64:128, tc_i * P : (tc_i + 1) * P],
                                QT_t[64:128, fc, :],
                                start=True,
                                stop=True,
                                tile_position=(64, 0),
                            )
                            e01 = ep.tile([P, 2 * NQ], bfh, tag="e01")
                            nc.scalar.activation(e01[:], s01[:], AF.Exp, scale=C**-0.5)
                            va = vap.tile([P, VW], bfh, tag="va")
                            nc.sync.dma_start(
                                va[:],
                                v_g[h_][tc_i // GT, tc_i % GT][
                                    :, fi * VW : (fi + 1) * VW
                                ],
                            )
                            pend[tc_i] = (e01, va)
                        if step >= SKEW:
                            tc_j = step - SKEW
                            e01, va = pend.pop(tc_j)
                            nc.tensor.matmul(
                                O0[:], va[:, 0 : HS + 1], e01[:, 0:NQ],
                                start=(tc_j == 0), stop=(tc_j == 15),
                            )
                            nc.tensor.matmul(
                                O1[:], va[:, HS + 1 : VW], e01[:, NQ : 2 * NQ],
                                start=(tc_j == 0), stop=(tc_j == 15),
                            )
                    for Oacc, col0 in ((O0, 0), (O1, 64)):
                        den = npool.tile([1, NQ], f32, tag="den")
                        nc.vector.tensor_copy(den[:], Oacc[HS : HS + 1, :])
                        rc = npool.tile([1, NQ], f32, tag="rc")
                        nc.vector.reciprocal_approx_fast(rc[:], den[:])
                        rch = npool.tile([1, NQ], bfh, tag="rch")
                        nc.vector.tensor_copy(rch[:], rc[:])
                        rb = rbp.tile([HS, NQ], f32, tag="rb")
                        nc.tensor.matmul(rb[:], onesw[:], rch[:], start=True, stop=True)
                        rbs = npool.tile([HS, NQ], f32, tag="rbs")
                        nc.vector.tensor_copy(rbs[:], rb[:])
                        dst = OT_t[col0 : col0 + HS, fc, :]
                        nc.vector.tensor_tensor(dst, Oacc[0:HS, :], rbs[:], op=Alu.mult)
                        nc.vector.tensor_scalar_add(
                            dst, dst, bias_t["bv"][col0 : col0 + HS, fc : fc + 1]
                        )

            if DBG:
                nc.sync.dma_start(dbg["ot"], OT_t[:])
                for h_ in range(2):
                    nc.sync.dma_start(dbg["kg"][h_], kT_g[h_][:])
                    nc.sync.dma_start(dbg["vg"][h_], v_g[h_][:])

            # ------------- Phase C: out-proj + residual + LN2 -------------
            outq_t = resC.tile([P, GT, C], f32, tag="outq")
            onT_t = resC.tile([P, CC, NQ], bfh, tag="onT")
            with ExitStack() as ph:
                lnp = ph.enter_context(tc.tile_pool(name="lnpC", bufs=2))
                trp = ph.enter_context(tc.tile_pool(name="trpC", bufs=3, space="PSUM"))
                mmp = ph.enter_context(tc.tile_pool(name="mmpC", bufs=3, space="PSUM"))
                evp = ph.enter_context(tc.tile_pool(name="evpC", bufs=3))

                for co in range(CC):
                    pm = mmp.tile([P, NQ], f32, tag="mmC")
                    for ci in range(CC):
                        nc.tensor.matmul(
                            pm[:],
                            wp_t[:, ci, co * P : (co + 1) * P],
                            OT_t[:, ci, :],
                            start=(ci == 0),
                            stop=(ci == CC - 1),
                        )
                    saT = evp.tile([P, NQ], f32, tag="saT")
                    nc.scalar.activation(
                        saT[:], pm[:], AF.Identity,
                        bias=bias_t["bp"][:, co : co + 1],
                    )
                    for qt in range(GT):
                        pt = trp.tile([P, P], f32, tag="trC")
                        nc.tensor.transpose(
                            pt[:], saT[:, qt * P : (qt + 1) * P], ident[:]
                        )
                        nc.vector.tensor_tensor(
                            outq_t[:, qt, co * P : (co + 1) * P],
                            pt[:],
                            xq_t[:, qt, co * P : (co + 1) * P],
                            op=Alu.add,
                        )
                if DBG:
                    nc.sync.dma_start(dbg["outq"], outq_t[:])
                # LN2 (token-major, input already in SBUF) -> feature-major onT
                for qt in range(GT):
                    xt = outq_t[:, qt, :]
                    stats = lnp.tile([P, 2, 6], f32, tag="ln_st6")
                    nc.vector.bn_stats(stats[:, 0, :], xt[:, 0:512])
                    nc.vector.bn_stats(stats[:, 1, :], xt[:, 512:1024])
                    mv = lnp.tile([P, 2], f32, tag="ln_mv")
                    nc.vector.bn_aggr(mv[:], stats[:])
                    st = lnp.tile([P, 1], f32, tag="ln_sd")
                    nc.scalar.activation(st[:], mv[:, 1:2], AF.Sqrt, bias=eps_t[:])
                    rs = lnp.tile([P, 1], f32, tag="ln_rs")
                    nc.vector.reciprocal(rs[:], st[:])
                    xn = lnp.tile([P, C], bfh, tag="ln_xn")
                    nc.vector.tensor_scalar(
                        xn[:], xt, mv[:, 0:1], rs[:],
                        op0=Alu.subtract, op1=Alu.mult,
                    )
                    for cc in range(CC):
                        pt = trp.tile([P, P], bfh, tag="trC")
                        nc.tensor.transpose(
                            pt[:], xn[:, cc * P : (cc + 1) * P], ident_h[:]
                        )
                        nc.vector.tensor_scalar(
                            onT_t[:, cc, qt * P : (qt + 1) * P],
                            pt[:],
                            bias_t["l2w"][:, cc : cc + 1],
                            bias_t["l2b"][:, cc : cc + 1],
                            op0=Alu.mult,
                            op1=Alu.add,
                        )

            # ---------------- Phase D: FFN ----------------
            with ExitStack() as ph:
                w1p = ph.enter_context(tc.tile_pool(name="w1p", bufs=3))
                w2p = ph.enter_context(tc.tile_pool(name="w2p", bufs=2))
                hp = ph.enter_context(tc.tile_pool(name="hp", bufs=1))
                mmph = ph.enter_context(tc.tile_pool(name="mmph", bufs=3, space="PSUM"))
                mmpy = ph.enter_context(tc.tile_pool(name="mmpy", bufs=2, space="PSUM"))
                trp = ph.enter_context(tc.tile_pool(name="trpD", bufs=2, space="PSUM"))
                evp = ph.enter_context(tc.tile_pool(name="evpD", bufs=3))
                finp = ph.enter_context(tc.tile_pool(name="finp", bufs=1))

                hT_t = hp.tile([P, FC, NQ], bfh, tag="hT")
                final_t = finp.tile([P, GT, C], f32, tag="final")

                for fc in range(FC):
                    w1c = w1p.tile([P, CC, P], bfh, tag="w1c")
                    nc.sync.dma_start(w1c[:], w1_d[fc])
                    pm = mmph.tile([P, NQ], f32, tag="mmh")
                    for cc in range(CC):
                        nc.tensor.matmul(
                            pm[:],
                            w1c[:, cc, :],
                            onT_t[:, cc, :],
                            start=(cc == 0),
                            stop=(cc == CC - 1),
                        )
                    nc.scalar.activation(
                        hT_t[:, fc, :], pm[:], AF.Gelu, bias=b1_t[:, fc : fc + 1]
                    )

                for co in range(CC):
                    w2c = w2p.tile([P, FC, P], bfh, tag="w2c")
                    nc.sync.dma_start(w2c[:], w2_d[co])
                    pm = mmpy.tile([P, NQ], f32, tag="mmy")
                    for fc in range(FC):
                        nc.tensor.matmul(
                            pm[:],
                            w2c[:, fc, :],
                            hT_t[:, fc, :],
                            start=(fc == 0),
                            stop=(fc == FC - 1),
                        )
                    yT = evp.tile([P, NQ], f32, tag="yT")
                    nc.scalar.activation(
                        yT[:], pm[:], AF.Identity,
                        bias=bias_t["b2"][:, co : co + 1],
                    )
                    for qt in range(GT):
                        pt = trp.tile([P, P], f32, tag="trD")
                        nc.tensor.transpose(
                            pt[:], yT[:, qt * P : (qt + 1) * P], ident[:]
                        )
                        nc.vector.tensor_tensor(
                            final_t[:, qt, co * P : (co + 1) * P],
                            pt[:],
                            outq_t[:, qt, co * P : (co + 1) * P],
                            op=Alu.add,
                        )
                nc.sync.dma_start(
                    y_d.rearrange("(q p) c -> p q c", p=P), final_t[:]
                )

    nc.compile()
    return nc


_NC_CACHE = None


def _get_program():
    global _NC_CACHE
    if _NC_CACHE is None:
        _NC_CACHE = build_program()
    return _NC_CACHE


import ml_dtypes

BF16 = ml_dtypes.bfloat16


def _merge_heads(w):
    # [H, C, HS] -> [C, H*HS]
    return np.ascontiguousarray(
        np.transpose(np.asarray(w), (1, 0, 2)).reshape(C, C).astype(BF16)
    )


def make_in_maps(inputs):
    x = np.ascontiguousarray(np.asarray(inputs["x"], dtype=np.float32))
    w1 = np.asarray(inputs["W1"], np.float32).astype(BF16)
    w2 = np.asarray(inputs["W2"], np.float32).astype(BF16)
    shared = {
        "wq": _merge_heads(inputs["Wq"]),
        "wk": _merge_heads(inputs["Wk"]),
        "wv": _merge_heads(inputs["Wv"]),
        "wp": np.ascontiguousarray(np.asarray(inputs["Wp"], np.float32).astype(BF16)),
        "w1p": np.ascontiguousarray(
            w1.reshape(CC, P, FC, P).transpose(2, 1, 0, 3)
        ),
        "w2p": np.ascontiguousarray(
            w2.reshape(FC, P, CC, P).transpose(2, 1, 0, 3)
        ),
        "bq": np.asarray(inputs["bq"], np.float32).reshape(C).copy(),
        "bk": np.asarray(inputs["bk"], np.float32).reshape(C).copy(),
        "bv": np.asarray(inputs["bv"], np.float32).reshape(C).copy(),
        "bp": np.asarray(inputs["bp"], np.float32).copy(),
        "b1": np.asarray(inputs["b1"], np.float32).copy(),
        "b2": np.asarray(inputs["b2"], np.float32).copy(),
        "l1w": np.asarray(inputs["ln1_w"], np.float32).copy(),
        "l1b": np.asarray(inputs["ln1_b"], np.float32).copy(),
        "l2w": np.asarray(inputs["ln2_w"], np.float32).copy(),
        "l2b": np.asarray(inputs["ln2_b"], np.float32).copy(),
    }
    in_maps = []
    for c in range(8):
        b, qs = c // 4, c % 4
        m = dict(shared)
        m["xq"] = np.ascontiguousarray(x[b, qs * NQ : (qs + 1) * NQ])
        in_maps.append(m)
    return in_maps


def kernel(**inputs):
    in_maps = make_in_maps(inputs)
    nc = _get_program()
    res = bass_utils.run_bass_kernel_spmd(nc, in_maps, core_ids=list(range(8)))
    out = np.empty((B, T, C), np.float32)
    for c in range(8):
        b, qs = c // 4, c % 4
        out[b, qs * NQ : (qs + 1) * NQ] = res.results[c]["y"]
    return out


# revision 23
# speedup vs baseline: 1.1302x; 1.0167x over previous
"""Trainium2 Bass kernel for a pre-norm transformer encoder block.

Problem: B=2, T=2048, C=1024, H=16 heads of 64, GELU FFN (4C), fp32.

Sharding: data-parallel over (batch, query-slice): 8 cores, core c handles
batch b=c//4 and token rows [(c%4)*512, (c%4+1)*512). Each core computes
LN1 + Q/K/V projections only for its OWN 512-token slice; K^T and the
ones-augmented V are exchanged among the 4 cores of a batch group with
pipelined DRAM AllGather collectives (K halves first so scores can start
before V lands). Attention/out-proj/FFN run on the core's 512 query rows.

All matmul operands are bf16 (fp32 PSUM accumulation); LN/softmax/residual
arithmetic stays fp32. Activations stay feature-major through the matmul
chain; PE transposes convert to/from token-major for LayerNorm. Softmax
runs over the partition axis with an ones-augmented V matmul producing
denominators for free; 1/denom via fast DVE reciprocal is broadcast across
partitions with a tiny ones-matmul on the PE (gpsimd stays free for the
collectives).
"""

import os
import sys

sys.path.insert(0, "/opt/trn_rl_repo")

import numpy as np

DBG = os.environ.get("BASSDBG", "0") == "1"

import concourse.bass as bass
import concourse.mybir as mybir
import concourse.tile as tile
from concourse import bacc, bass_utils
from concourse.masks import make_identity

P = 128
B, T, C, H = 2, 2048, 1024, 16
HS = C // H  # 64
F = 4 * C  # 4096
NQ = 512  # token rows per core
CC = C // P  # 8
FC = F // P  # 32
GT = NQ // P  # 4 token tiles per slice
VW = 2 * (HS + 1)  # 130: ones-augmented V columns per head pair
EPS = 1e-5

f32 = mybir.dt.float32
bfh = mybir.dt.bfloat16
AF = mybir.ActivationFunctionType
Alu = mybir.AluOpType

GROUPS = [[0, 1, 2, 3], [4, 5, 6, 7]]


def build_program():
    nc = bacc.Bacc("TRN2", target_bir_lowering=False, debug=False, num_devices=8)

    xq_d = nc.dram_tensor("xq", [NQ, C], f32, kind="ExternalInput").ap()
    wq_d = nc.dram_tensor("wq", [C, C], bfh, kind="ExternalInput").ap()
    wk_d = nc.dram_tensor("wk", [C, C], bfh, kind="ExternalInput").ap()
    wv_d = nc.dram_tensor("wv", [C, C], bfh, kind="ExternalInput").ap()
    wp_d = nc.dram_tensor("wp", [C, C], bfh, kind="ExternalInput").ap()
    w1_d = nc.dram_tensor("w1p", [FC, P, CC, P], bfh, kind="ExternalInput").ap()
    w2_d = nc.dram_tensor("w2p", [CC, P, FC, P], bfh, kind="ExternalInput").ap()
    bias_names = ["l1w", "l1b", "bk", "bq", "bv", "bp", "l2w", "l2b", "b2"]
    bias_d = {
        n: nc.dram_tensor(n, [C], f32, kind="ExternalInput").ap() for n in bias_names
    }
    b1_d = nc.dram_tensor("b1", [F], f32, kind="ExternalInput").ap()
    y_d = nc.dram_tensor("y", [NQ, C], f32, kind="ExternalOutput").ap()
    dbg = {}
    if DBG:
        dbg["xnq"] = nc.dram_tensor("d_xnq", [P, CC, NQ], bfh, kind="ExternalOutput").ap()
        dbg["qt"] = nc.dram_tensor("d_qt", [P, CC, NQ], bfh, kind="ExternalOutput").ap()
        dbg["kg"] = nc.dram_tensor("d_kg", [2, 4, GT, P, NQ], bfh, kind="ExternalOutput").ap()
        dbg["vg"] = nc.dram_tensor("d_vg", [2, 4, GT, P, 8 * (HS + 1)], bfh, kind="ExternalOutput").ap()
        dbg["ot"] = nc.dram_tensor("d_ot", [P, CC, NQ], bfh, kind="ExternalOutput").ap()
        dbg["outq"] = nc.dram_tensor("d_outq", [P, GT, C], f32, kind="ExternalOutput").ap()

    with tile.TileContext(nc) as tc:
        from contextlib import ExitStack

        with ExitStack() as top:
            const = top.enter_context(tc.tile_pool(name="const", bufs=1))
            ident = const.tile([P, P], f32)
            make_identity(nc, ident[:])
            ident_h = const.tile([P, P], bfh, tag="ident_h")
            make_identity(nc, ident_h[:])
            eps_t = const.tile([P, 1], f32, tag="eps")
            nc.vector.memset(eps_t[:], EPS)
            ones_f = const.tile([P, 1], f32, tag="ones_f")
            nc.vector.memset(ones_f[:], 1.0)
            ones_r = const.tile([P, 1], bfh, tag="ones_r")
            nc.vector.tensor_copy(ones_r[:], ones_f[:])
            onesw_f = const.tile([1, HS], f32, tag="onesw_f")
            nc.vector.memset(onesw_f[:], 1.0)
            onesw = const.tile([1, HS], bfh, tag="onesw")
            nc.vector.tensor_copy(onesw[:], onesw_f[:])

            # ---- bias loads: contiguous [o, P] staging + PE transpose ----
            bias_t = {}
            with ExitStack() as bctx:
                bsp = bctx.enter_context(tc.tile_pool(name="bstage", bufs=2))
                btp = bctx.enter_context(
                    tc.tile_pool(name="btp", bufs=2, space="PSUM")
                )
                for n in bias_names:
                    st = bsp.tile([CC, P], f32, tag="bst")
                    nc.sync.dma_start(st[:], bias_d[n].rearrange("(o p) -> o p", p=P))
                    pm = btp.tile([P, CC], f32, tag="btr")
                    nc.tensor.matmul(
                        pm[:], st[:], ident[0:CC, 0:CC], start=True, stop=True
                    )
                    bt = const.tile([P, CC], f32, tag=f"bias_{n}")
                    nc.scalar.copy(bt[:], pm[:])
                    bias_t[n] = bt
                st = bsp.tile([FC, P], f32, tag="bst32")
                nc.sync.dma_start(st[:], b1_d.rearrange("(o p) -> o p", p=P))
                pm = btp.tile([P, FC], f32, tag="btr32")
                nc.tensor.matmul(
                    pm[:], st[:], ident[0:FC, 0:FC], start=True, stop=True
                )
                b1_t = const.tile([P, FC], f32, tag="bias_b1")
                nc.scalar.copy(b1_t[:], pm[:])

            dramp = top.enter_context(tc.tile_pool(name="dscratch", bufs=1, space="DRAM"))
            warm_in = dramp.tile([P, 4], f32, name="warm_in", tag="warm_in")
            warm_out = dramp.tile([4, P, 4], f32, name="warm_out", tag="warm_out")
            warm_sb = const.tile([P, 4], f32, tag="warm_sb")
            nc.vector.memset(warm_sb[:], 0.0)
            nc.sync.dma_start(warm_in[:], warm_sb[:])
            nc.gpsimd.collective_compute(
                "AllGather",
                Alu.bypass,
                replica_groups=GROUPS,
                ins=[warm_in.opt()],
                outs=[warm_out.opt()],
            )
            VH = 8 * (HS + 1)  # 520
            kT_in = [
                dramp.tile([GT, P, NQ], bfh, name=f"kTin{h_}", tag=f"kTin{h_}")
                for h_ in range(2)
            ]
            v_in = [
                dramp.tile([GT, P, VH], bfh, name=f"vin{h_}", tag=f"vin{h_}")
                for h_ in range(2)
            ]
            kT_g = [
                dramp.tile([4, GT, P, NQ], bfh, name=f"kTg{h_}", tag=f"kTg{h_}")
                for h_ in range(2)
            ]
            v_g = [
                dramp.tile([4, GT, P, VH], bfh, name=f"vg{h_}", tag=f"vg{h_}")
                for h_ in range(2)
            ]

            res = top.enter_context(tc.tile_pool(name="resident", bufs=1))
            QT_t = res.tile([P, CC, NQ], bfh, tag="QT")

            # ---------------- Phase A: LN1 + Q/K/V projections ----------------
            with ExitStack() as ph:
                lnp = ph.enter_context(tc.tile_pool(name="lnp", bufs=4))
                trp = ph.enter_context(tc.tile_pool(name="trp", bufs=3, space="PSUM"))
                mmp = ph.enter_context(tc.tile_pool(name="mmpA", bufs=4, space="PSUM"))
                evp = ph.enter_context(tc.tile_pool(name="evpA", bufs=3))
                wgp = ph.enter_context(tc.tile_pool(name="wgp", bufs=1))

                xnq = res.tile([P, CC, NQ], bfh, tag="xnT")

                wk_t = wgp.tile([P, CC, C], bfh, tag="wk")
                nc.sync.dma_start(wk_t[:], wk_d.rearrange("(o p) f -> p o f", p=P))
                wv_t = wgp.tile([P, CC, C], bfh, tag="wv")
                nc.sync.dma_start(wv_t[:], wv_d.rearrange("(o p) f -> p o f", p=P))
                # LN1 over own slice -> xnq (feature-major, bf16, ln1 applied)
                inv_c = 1.0 / C
                for tt in range(GT):
                    xt = lnp.tile([P, C], f32, tag="ln_x")
                    nc.sync.dma_start(xt[:], xq_d[tt * P : (tt + 1) * P, :])
                    stats = lnp.tile([P, 2, 6], f32, tag="ln_st6")
                    nc.vector.bn_stats(stats[:, 0, :], xt[:, 0:512])
                    nc.vector.bn_stats(stats[:, 1, :], xt[:, 512:1024])
                    mv = lnp.tile([P, 2], f32, tag="ln_mv")
                    nc.vector.bn_aggr(mv[:], stats[:])
                    st = lnp.tile([P, 1], f32, tag="ln_sd")
                    nc.scalar.activation(
                        st[:], mv[:, 1:2], AF.Sqrt, bias=eps_t[:]
                    )
                    rs = lnp.tile([P, 1], f32, tag="ln_rs")
                    nc.vector.reciprocal(rs[:], st[:])
                    xn = lnp.tile([P, C], bfh, tag="ln_xn")
                    nc.vector.tensor_scalar(
                        xn[:], xt[:], mv[:, 0:1], rs[:],
                        op0=Alu.subtract, op1=Alu.mult,
                    )
                    for cc in range(CC):
                        pt = trp.tile([P, P], bfh, tag="ln_tr")
                        nc.tensor.transpose(pt[:], xn[:, cc * P : (cc + 1) * P], ident_h[:])
                        nc.vector.tensor_scalar(
                            xnq[:, cc, tt * P : (tt + 1) * P],
                            pt[:],
                            bias_t["l1w"][:, cc : cc + 1],
                            bias_t["l1b"][:, cc : cc + 1],
                            op0=Alu.mult,
                            op1=Alu.add,
                        )

                # K projection (own slice) for half h_ -> kv_in cols 0:512
                def k_half(h_):
                    for fi in range(4):
                        fc = h_ * 4 + fi
                        pm = mmp.tile([P, NQ], f32, tag="mmA")
                        for cc in range(CC):
                            nc.tensor.matmul(
                                pm[:],
                                wk_t[:, cc, fc * P : (fc + 1) * P],
                                xnq[:, cc, :],
                                start=(cc == 0),
                                stop=(cc == CC - 1),
                            )
                        ev = evp.tile([P, NQ], bfh, tag="kev")
                        nc.scalar.activation(
                            ev[:], pm[:], AF.Identity,
                            bias=bias_t["bk"][:, fc : fc + 1],
                        )
                        nc.sync.dma_start(kT_in[h_][fi], ev[:])

                # V projection (own slice), ones-augmented -> kv_in cols 512:1032
                def v_half(h_):
                    for tt in range(GT):
                        pm = mmp.tile([P, NQ], f32, tag="mmA")
                        for cc in range(CC):
                            nc.tensor.matmul(
                                pm[:],
                                xnq[:, cc, tt * P : (tt + 1) * P],
                                wv_t[:, cc, h_ * 512 : (h_ + 1) * 512],
                                start=(cc == 0),
                                stop=(cc == CC - 1),
                            )
                        ev = evp.tile([P, 8, HS + 1], bfh, tag="vev")
                        nc.scalar.copy(
                            ev[:, :, 0:HS],
                            pm[:].rearrange("p (h d) -> p h d", d=HS),
                        )
                        nc.vector.memset(ev[:, :, HS : HS + 1], 1.0)
                        nc.sync.dma_start(v_in[h_][tt], ev[:])

                def fire(buf_in, buf_out):
                    nc.gpsimd.collective_compute(
                        "AllGather",
                        Alu.bypass,
                        replica_groups=GROUPS,
                        ins=[buf_in.opt()],
                        outs=[buf_out.opt()],
                    )

                k_half(0)
                fire(kT_in[0], kT_g[0])
                wq_t = wgp.tile([P, CC, C], bfh, tag="wq")
                nc.sync.dma_start(wq_t[:], wq_d.rearrange("(o p) f -> p o f", p=P))
                v_half(0)
                fire(v_in[0], v_g[0])
                k_half(1)
                fire(kT_in[1], kT_g[1])
                v_half(1)
                fire(v_in[1], v_g[1])

                # Q projection from own slice
                for fc in range(CC):
                    pm = mmp.tile([P, NQ], f32, tag="mmA")
                    for cc in range(CC):
                        nc.tensor.matmul(
                            pm[:],
                            wq_t[:, cc, fc * P : (fc + 1) * P],
                            xnq[:, cc, :],
                            start=(cc == 0),
                            stop=(cc == CC - 1),
                        )
                    nc.scalar.activation(
                        QT_t[:, fc, :], pm[:], AF.Identity,
                        bias=bias_t["bq"][:, fc : fc + 1],
                    )
                if DBG:
                    nc.sync.dma_start(dbg["xnq"], xnq[:])
                    nc.sync.dma_start(dbg["qt"], QT_t[:])

            # ---------------- Phase B: attention (per head pair) ----------------
            resC = top.enter_context(tc.tile_pool(name="resC", bufs=1))
            wp_t = resC.tile([P, CC, C], bfh, tag="wp")
            nc.sync.dma_start(wp_t[:], wp_d.rearrange("(o p) f -> p o f", p=P))
            xq_t = resC.tile([P, GT, C], f32, tag="xqt")
            nc.sync.dma_start(xq_t[:], xq_d.rearrange("(q p) c -> p q c", p=P))

            with ExitStack() as ph:
                kp = ph.enter_context(tc.tile_pool(name="kp", bufs=2))
                vap = ph.enter_context(tc.tile_pool(name="vap", bufs=12))
                ep = ph.enter_context(tc.tile_pool(name="ep", bufs=32))
                sp = ph.enter_context(tc.tile_pool(name="sp", bufs=2, space="PSUM"))
                op_ = ph.enter_context(tc.tile_pool(name="op", bufs=2, space="PSUM"))
                rbp = ph.enter_context(tc.tile_pool(name="rbp", bufs=2, space="PSUM"))
                npool = ph.enter_context(tc.tile_pool(name="npool", bufs=4))
                OT_t = res.tile([P, CC, NQ], bfh, tag="OT")

                for fc in range(CC):  # head pair (2*fc, 2*fc+1)
                    h_ = fc // 4
                    fi = fc % 4
                    kT_pair = kp.tile([P, T], bfh, tag="kT")
                    for slot in range(4):
                        nc.sync.dma_start(
                            kT_pair[:, slot * NQ : (slot + 1) * NQ],
                            kT_g[h_][slot, fi],
                        )
                    O0 = op_.tile([HS + 1, NQ], f32, tag="Oacc")
                    O1 = op_.tile([HS + 1, NQ], f32, tag="Oacc")
                    SKEW = 4
                    pend = {}
                    for step in range(16 + SKEW):
                        if step < 16:
                            tc_i = step
                            s01 = sp.tile([P, 2 * NQ], f32, tag="sc")
                            nc.tensor.matmul(
                                s01[:, 0:NQ],
                                kT_pair[0:64, tc_i * P : (tc_i + 1) * P],
                                QT_t[0:64, fc, :],
                                start=True,
                                stop=True,
                            )
                            nc.tensor.matmul(
                                s01[:, NQ : 2 * NQ],
                                kT_pair[64:128, tc_i * P : (tc_i + 1) * P],
                                QT_t[64:128, fc, :],
                                start=True,
                                stop=True,
                                tile_position=(64, 0),
                            )
                            e01 = ep.tile([P, 2 * NQ], bfh, tag="e01")
                            nc.scalar.activation(e01[:], s01[:], AF.Exp, scale=C**-0.5)
                            va = vap.tile([P, VW], bfh, tag="va")
                            nc.sync.dma_start(
                                va[:],
                                v_g[h_][tc_i // GT, tc_i % GT][
                                    :, fi * VW : (fi + 1) * VW
                                ],
                            )
                            pend[tc_i] = (e01, va)
                        if step >= SKEW:
                            tc_j = step - SKEW
                            e01, va = pend.pop(tc_j)
                            nc.tensor.matmul(
                                O0[:], va[:, 0 : HS + 1], e01[:, 0:NQ],
                                start=(tc_j == 0), stop=(tc_j == 15),
                            )
                            nc.tensor.matmul(
                                O1[:], va[:, HS + 1 : VW], e01[:, NQ : 2 * NQ],
                                start=(tc_j == 0), stop=(tc_j == 15),
                            )
                    for Oacc, col0 in ((O0, 0), (O1, 64)):
                        den = npool.tile([1, NQ], f32, tag="den")
                        nc.vector.tensor_copy(den[:], Oacc[HS : HS + 1, :])
                        rc = npool.tile([1, NQ], f32, tag="rc")
                        nc.vector.reciprocal_approx_fast(rc[:], den[:])
                        rch = npool.tile([1, NQ], bfh, tag="rch")
                        nc.vector.tensor_copy(rch[:], rc[:])
                        rb = rbp.tile([HS, NQ], f32, tag="rb")
                        nc.tensor.matmul(rb[:], onesw[:], rch[:], start=True, stop=True)
                        rbs = npool.tile([HS, NQ], f32, tag="rbs")
                        nc.vector.tensor_copy(rbs[:], rb[:])
                        dst = OT_t[col0 : col0 + HS, fc, :]
                        nc.vector.tensor_tensor(dst, Oacc[0:HS, :], rbs[:], op=Alu.mult)
                        nc.vector.tensor_scalar_add(
                            dst, dst, bias_t["bv"][col0 : col0 + HS, fc : fc + 1]
                        )

            if DBG:
                nc.sync.dma_start(dbg["ot"], OT_t[:])
                for h_ in range(2):
                    nc.sync.dma_start(dbg["kg"][h_], kT_g[h_][:])
                    nc.sync.dma_start(dbg["vg"][h_], v_g[h_][:])

            # ------------- Phase C: out-proj + residual + LN2 -------------
            outq_t = resC.tile([P, GT, C], f32, tag="outq")
            onT_t = resC.tile([P, CC, NQ], bfh, tag="onT")
            with ExitStack() as ph:
                lnp = ph.enter_context(tc.tile_pool(name="lnpC", bufs=2))
                trp = ph.enter_context(tc.tile_pool(name="trpC", bufs=3, space="PSUM"))
                mmp = ph.enter_context(tc.tile_pool(name="mmpC", bufs=3, space="PSUM"))
                evp = ph.enter_context(tc.tile_pool(name="evpC", bufs=3))

                for co in range(CC):
                    pm = mmp.tile([P, NQ], f32, tag="mmC")
                    for ci in range(CC):
                        nc.tensor.matmul(
                            pm[:],
                            wp_t[:, ci, co * P : (co + 1) * P],
                            OT_t[:, ci, :],
                            start=(ci == 0),
                            stop=(ci == CC - 1),
                        )
                    saT = evp.tile([P, NQ], f32, tag="saT")
                    nc.scalar.activation(
                        saT[:], pm[:], AF.Identity,
                        bias=bias_t["bp"][:, co : co + 1],
                    )
                    for qt in range(GT):
                        pt = trp.tile([P, P], f32, tag="trC")
                        nc.tensor.transpose(
                            pt[:], saT[:, qt * P : (qt + 1) * P], ident[:]
                        )
                        nc.vector.tensor_tensor(
                            outq_t[:, qt, co * P : (co + 1) * P],
                            pt[:],
                            xq_t[:, qt, co * P : (co + 1) * P],
                            op=Alu.add,
                        )
                if DBG:
                    nc.sync.dma_start(dbg["outq"], outq_t[:])
                # LN2 (token-major, input already in SBUF) -> feature-major onT
                for qt in range(GT):
                    xt = outq_t[:, qt, :]
                    stats = lnp.tile([P, 2, 6], f32, tag="ln_st6")
                    nc.vector.bn_stats(stats[:, 0, :], xt[:, 0:512])
                    nc.vector.bn_stats(stats[:, 1, :], xt[:, 512:1024])
                    mv = lnp.tile([P, 2], f32, tag="ln_mv")
                    nc.vector.bn_aggr(mv[:], stats[:])
                    st = lnp.tile([P, 1], f32, tag="ln_sd")
                    nc.scalar.activation(st[:], mv[:, 1:2], AF.Sqrt, bias=eps_t[:])
                    rs = lnp.tile([P, 1], f32, tag="ln_rs")
                    nc.vector.reciprocal(rs[:], st[:])
                    xn = lnp.tile([P, C], bfh, tag="ln_xn")
                    nc.vector.tensor_scalar(
                        xn[:], xt, mv[:, 0:1], rs[:],
                        op0=Alu.subtract, op1=Alu.mult,
                    )
                    for cc in range(CC):
                        pt = trp.tile([P, P], bfh, tag="trC")
                        nc.tensor.transpose(
                            pt[:], xn[:, cc * P : (cc + 1) * P], ident_h[:]
                        )
                        nc.vector.tensor_scalar(
                            onT_t[:, cc, qt * P : (qt + 1) * P],
                            pt[:],
                            bias_t["l2w"][:, cc : cc + 1],
                            bias_t["l2b"][:, cc : cc + 1],
                            op0=Alu.mult,
                            op1=Alu.add,
                        )

            # ---------------- Phase D: FFN ----------------
            with ExitStack() as ph:
                w1p = ph.enter_context(tc.tile_pool(name="w1p", bufs=3))
                w2p = ph.enter_context(tc.tile_pool(name="w2p", bufs=2))
                hp = ph.enter_context(tc.tile_pool(name="hp", bufs=1))
                mmph = ph.enter_context(tc.tile_pool(name="mmph", bufs=3, space="PSUM"))
                mmpy = ph.enter_context(tc.tile_pool(name="mmpy", bufs=2, space="PSUM"))
                trp = ph.enter_context(tc.tile_pool(name="trpD", bufs=2, space="PSUM"))
                evp = ph.enter_context(tc.tile_pool(name="evpD", bufs=3))
                finp = ph.enter_context(tc.tile_pool(name="finp", bufs=1))

                hT_t = hp.tile([P, FC, NQ], bfh, tag="hT")
                final_t = finp.tile([P, GT, C], f32, tag="final")

                for fc in range(FC):
                    w1c = w1p.tile([P, CC, P], bfh, tag="w1c")
                    nc.sync.dma_start(w1c[:], w1_d[fc])
                    pm = mmph.tile([P, NQ], f32, tag="mmh")
                    for cc in range(CC):
                        nc.tensor.matmul(
                            pm[:],
                            w1c[:, cc, :],
                            onT_t[:, cc, :],
                            start=(cc == 0),
                            stop=(cc == CC - 1),
                        )
                    nc.scalar.activation(
                        hT_t[:, fc, :], pm[:], AF.Gelu, bias=b1_t[:, fc : fc + 1]
                    )

                for co in range(CC):
                    w2c = w2p.tile([P, FC, P], bfh, tag="w2c")
                    nc.sync.dma_start(w2c[:], w2_d[co])
                    pm = mmpy.tile([P, NQ], f32, tag="mmy")
                    for fc in range(FC):
                        nc.tensor.matmul(
                            pm[:],
                            w2c[:, fc, :],
                            hT_t[:, fc, :],
                            start=(fc == 0),
                            stop=(fc == FC - 1),
                        )
                    yT = evp.tile([P, NQ], f32, tag="yT")
                    nc.scalar.activation(
                        yT[:], pm[:], AF.Identity,
                        bias=bias_t["b2"][:, co : co + 1],
                    )
                    for qt in range(GT):
                        pt = trp.tile([P, P], f32, tag="trD")
                        nc.tensor.transpose(
                            pt[:], yT[:, qt * P : (qt + 1) * P], ident[:]
                        )
                        nc.vector.tensor_tensor(
                            final_t[:, qt, co * P : (co + 1) * P],
                            pt[:],
                            outq_t[:, qt, co * P : (co + 1) * P],
                            op=Alu.add,
                        )
                nc.sync.dma_start(
                    y_d.rearrange("(q p) c -> p q c", p=P), final_t[:]
                )

    nc.compile()
    return nc


_NC_CACHE = None


def _get_program():
    global _NC_CACHE
    if _NC_CACHE is None:
        _NC_CACHE = build_program()
    return _NC_CACHE


import ml_dtypes

BF16 = ml_dtypes.bfloat16


def _merge_heads(w):
    # [H, C, HS] -> [C, H*HS]
    return np.ascontiguousarray(
        np.transpose(np.asarray(w), (1, 0, 2)).reshape(C, C).astype(BF16)
    )


def make_in_maps(inputs):
    x = np.ascontiguousarray(np.asarray(inputs["x"], dtype=np.float32))
    w1 = np.asarray(inputs["W1"], np.float32).astype(BF16)
    w2 = np.asarray(inputs["W2"], np.float32).astype(BF16)
    shared = {
        "wq": _merge_heads(inputs["Wq"]),
        "wk": _merge_heads(inputs["Wk"]),
        "wv": _merge_heads(inputs["Wv"]),
        "wp": np.ascontiguousarray(np.asarray(inputs["Wp"], np.float32).astype(BF16)),
        "w1p": np.ascontiguousarray(
            w1.reshape(CC, P, FC, P).transpose(2, 1, 0, 3)
        ),
        "w2p": np.ascontiguousarray(
            w2.reshape(FC, P, CC, P).transpose(2, 1, 0, 3)
        ),
        "bq": np.asarray(inputs["bq"], np.float32).reshape(C).copy(),
        "bk": np.asarray(inputs["bk"], np.float32).reshape(C).copy(),
        "bv": np.asarray(inputs["bv"], np.float32).reshape(C).copy(),
        "bp": np.asarray(inputs["bp"], np.float32).copy(),
        "b1": np.asarray(inputs["b1"], np.float32).copy(),
        "b2": np.asarray(inputs["b2"], np.float32).copy(),
        "l1w": np.asarray(inputs["ln1_w"], np.float32).copy(),
        "l1b": np.asarray(inputs["ln1_b"], np.float32).copy(),
        "l2w": np.asarray(inputs["ln2_w"], np.float32).copy(),
        "l2b": np.asarray(inputs["ln2_b"], np.float32).copy(),
    }
    in_maps = []
    for c in range(8):
        b, qs = c // 4, c % 4
        m = dict(shared)
        m["xq"] = np.ascontiguousarray(x[b, qs * NQ : (qs + 1) * NQ])
        in_maps.append(m)
    return in_maps


def kernel(**inputs):
    in_maps = make_in_maps(inputs)
    nc = _get_program()
    res = bass_utils.run_bass_kernel_spmd(nc, in_maps, core_ids=list(range(8)))
    out = np.empty((B, T, C), np.float32)
    for c in range(8):
        b, qs = c // 4, c % 4
        out[b, qs * NQ : (qs + 1) * NQ] = res.results[c]["y"]
    return out
